# revision 22
# baseline (speedup 1.0000x reference)
"""Trainium2 Bass kernel for scatter(w_est -> W[rows, cols]) followed by X @ W.

Strategy (data-parallel over rows, 8 NeuronCores):
  - Host: scatter w_est into W (256x256) - tiny, and numpy assignment matches
    the reference's last-write-wins scatter semantics.
  - Host: shard X row-wise into 8 shards of 62500 rows; transpose each shard
    to feature-major [256, rows] (TensorE contracts over the partition dim)
    (no row padding: 62500 = 122 * 512 + 36).
  - Precision: the harness gate is ||err||/||ref|| < 2e-2; a single fp16
    product (X16 @ W16, fp32 PSUM accumulate, fp16 output) measures ~4e-4,
    so no hi/lo splitting is needed. That halves HBM traffic vs the hi/lo
    kernel: 32 MB in + 32 MB out per core instead of 64+64, and the kernel
    is HBM-bound (~358 GB/s/core), so time ~= bytes / 358 GB/s.
  - Device (per core): outT[m-chunk, blk] = sum over k-chunks of
    W[k,m].T @ XT[k, blk] for 512-row blocks; 8 PSUM banks in flight
    (2 m-chunks x 4 blocks), bank-major so banks recycle early. PSUM->SBUF
    copies downcast to fp16 on the DVE. Input DMAs (sync ring) and output
    DMAs (scalar ring) move 1 MiB per descriptor batch (8 KB per partition
    line), which is the >=75%-of-peak regime for the SDMA engines.
  - Host: transpose each core's outT back, upcast to fp32, concatenate.
"""

import numpy as np

N_ROWS = 500000
D = 256
N_CORES = 8
RPC = N_ROWS // N_CORES            # 62500 rows per core
BLK = 512                          # rows per matmul (moving free dim)
# 122 blocks of 512 rows + one 36-row tail block - zero padding. DMA
# descriptor lines span a whole chunk (>=6 KB even for the tail chunk), so
# the odd block width costs nothing. Chunks: a small first chunk so the PE
# starts early, then 8-block chunks = 1 MiB per DMA stream (the >=75%-of-
# peak regime for the SDMA engines; 2 MiB chunks and sub-MiB tail chunks
# both measured slower).
BLOCKS = [BLK] * 122 + [RPC - 122 * BLK]       # [512]*122 + [36]
RPC_PAD = sum(BLOCKS)              # 62500 - no padding
# 16-block middle chunks keep the fp8 input stream at 8 KB per descriptor
# line / 1 MiB per transfer (4 KB lines measured only ~338 GB/s); small
# first/last chunks keep pipeline fill and drain short.
CHUNKS = [BLOCKS[:4]] + [BLOCKS[4 + 16 * i:4 + 16 * (i + 1)]
                         for i in range(7)] + [BLOCKS[116:120],
                                               BLOCKS[120:]]

_CACHE = {}
LAST_RESULT = None  # BassKernelResults of the most recent run (for profiling)


def _build():
    import concourse.tile as tile
    from concourse import bacc, mybir

    DT = mybir.dt.float16
    DT8 = mybir.dt.float8e3   # e3m4: 4 mantissa bits, range +-15.5
    nc = bacc.Bacc("TRN2", target_bir_lowering=False, debug=False,
                   num_devices=N_CORES)
    # X ships as fp8 e3m4 (measured 1.3e-2 rel err vs the 2e-2 gate with W
    # kept fp16) - halves the input stream so DMA drops below the PE/DVE
    # poles of the ridge.
    xt = nc.dram_tensor("xt", [D, RPC_PAD], DT8, kind="ExternalInput").ap()
    # wp[:, i*128:(i+1)*128] = W[k*128:(k+1)*128, m*128:(m+1)*128], i = 2k+m
    wp = nc.dram_tensor("wp", [128, 512], DT, kind="ExternalInput").ap()
    outT = nc.dram_tensor("outT", [D, RPC_PAD], DT, kind="ExternalOutput").ap()

    with tile.TileContext(nc) as tc:
        with tc.tile_pool(name="wpool", bufs=1) as wpool, \
             tc.tile_pool(name="xpool", bufs=3) as xpool, \
             tc.tile_pool(name="opool", bufs=3) as opool, \
             tc.psum_pool(name="pspool", bufs=1) as pspool:
            # W load goes on the scalar HWDGE ring so it doesn't delay the
            # first X chunk on the sync ring.
            wt = wpool.tile([128, 512], DT, name="w", tag="w")
            nc.scalar.dma_start(wt[:], wp[:, :])

            c0 = 0
            g_idx = 0                          # global PSUM-group counter
            pending = []                       # deferred output DMA args
            for widths in CHUNKS:
                ccols = sum(widths)
                x = []
                for k in range(2):
                    t = xpool.tile([128, ccols], DT8, name=f"x{k}",
                                   tag=f"x{k}")
                    nc.sync.dma_start(
                        t[:], xt[k * 128:(k + 1) * 128, c0:c0 + ccols])
                    x.append(t)
                # Output DMAs also go on the sync ring (the scalar/ACT
                # engine now runs copies, and a dma_start's semaphore wait
                # would stall whichever sequencer issues it). Emission is
                # deferred TWO chunks so the copy-completion wait sits
                # behind two chunks' worth of input dma_starts in the sync
                # FIFO - one-chunk deferral measured 25 us of input
                # starvation with 16-block chunks.
                if len(pending) >= 2:
                    for args in pending.pop(0):
                        nc.sync.dma_start(*args)
                st = [opool.tile([128, ccols], DT, name=f"st{m}",
                                 tag=f"st{m}") for m in range(2)]

                gi = 0                         # block index into chunk
                gcol = 0                       # col offset into chunk
                while gi < len(widths):
                    grp = widths[gi:gi + 4]    # blocks in this PSUM group
                    gcols = sum(grp)
                    for m in range(2):
                        # one 4-bank PSUM tile per (group, m): matmuls
                        # accumulate into bank-aligned 512-col slices,
                        # then ONE wide copy drains the whole tile. The
                        # DVE's PSUM read path caps at ~0.95 elem/ns, so
                        # ~1/4 of the copies go to the otherwise-idle ACT
                        # engine to keep the copy stream off the critical
                        # path.
                        ps = pspool.tile([128, gcols], mybir.dt.float32,
                                         name=f"ps{m}", tag=f"ps{m}")
                        bc = 0
                        for w in grp:
                            for k in range(2):
                                nc.tensor.matmul(
                                    ps[:, bc:bc + w],
                                    wt[:, (2 * k + m) * 128:
                                          (2 * k + m + 1) * 128],
                                    x[k][:, gcol + bc:gcol + bc + w],
                                    start=(k == 0), stop=(k == 1))
                            bc += w
                        dst = st[m][:, gcol:gcol + gcols]
                        if m == 1 and g_idx % 2 == 0:
                            nc.scalar.copy(dst, ps[:])
                        else:
                            nc.vector.tensor_scalar_mul(dst, ps[:], 1.0)
                    g_idx += 1
                    gi += len(grp)
                    gcol += gcols
                pending.append(
                    [(outT[m * 128:(m + 1) * 128, c0:c0 + ccols],
                      st[m][:]) for m in range(2)])
                c0 += ccols
            for plist in pending:
                for args in plist:
                    nc.sync.dma_start(*args)

    nc.compile()
    return nc


def kernel(X, w_est, rows, cols):
    global LAST_RESULT
    from concourse.bass_utils import run_bass_kernel_spmd

    X = np.asarray(X, dtype=np.float32)
    w_est = np.asarray(w_est, dtype=np.float32)
    rows = np.asarray(rows)
    cols = np.asarray(cols)

    W = np.zeros((D, D), dtype=np.float32)
    W[rows, cols] = w_est  # last-write-wins, same as XLA scatter-set

    if "nc" not in _CACHE:
        _CACHE["nc"] = _build()
    nc = _CACHE["nc"]

    W16 = W.astype(np.float16)
    wp = np.concatenate([W16[0:128, 0:128], W16[0:128, 128:256],
                         W16[128:256, 0:128], W16[128:256, 128:256]], axis=1)
    wp = np.ascontiguousarray(wp)

    import ml_dtypes
    f8 = ml_dtypes.float8_e3m4
    in_maps = []
    for c in range(N_CORES):
        xt = np.zeros((D, RPC_PAD), dtype=f8)
        xt[:, :RPC] = X[c * RPC:(c + 1) * RPC].T.astype(f8)
        in_maps.append({"xt": xt, "wp": wp})

    # the axon-tunneled device occasionally reports a transient
    # NRT_EXEC_UNIT_UNRECOVERABLE on the first run after another process
    # used it. A plain retry reuses the poisoned PJRT client and keeps
    # failing, so drop the jax backends (fresh client re-opens the device)
    # and ask the runtime to reset the cores before retrying.
    last_exc = None
    for attempt in range(4):
        try:
            res = run_bass_kernel_spmd(nc, in_maps,
                                       core_ids=list(range(N_CORES)))
            break
        except Exception as e:
            last_exc = e
            import os
            import time
            os.environ["NEURON_RT_RESET_CORES"] = "1"
            try:
                import jax
                jax.extend.backend.clear_backends()
            except Exception:
                pass
            time.sleep(10.0 * (attempt + 1))
    else:
        raise last_exc
    LAST_RESULT = res
    return np.concatenate(
        [r["outT"][:, :RPC].T.astype(np.float32) for r in res.results],
        axis=0)
